# revision 9
# baseline (speedup 1.0000x reference)
"""LocalRNN (windowed LSTM) Trainium2 kernel.

Problem: x (8, 2048, 128); for every position s, run a W=16-step LSTM over
x[b, s-15 .. s] (zero-padded) with h0=c0=0; output the final hidden state.

Sharding: batch across the 8 cores (core c handles batch c; windows never
cross batches, so no halo is needed).

Per-core layout is feature-major ("transposed"): hidden dim d=128 lives on
SBUF partitions, positions on the free dim.  Per step and 512-position chunk:

  psum[d, 4*512] = whh_j @ h  (+)  I @ xg_j_slice      (fp32r matmuls, PSUM acc)
  s = sigmoid(psum)                                    (one ACT pass, 4 banks)
  u = (s_g - 0.5) * s_i            (DVE fused scalar_tensor_tensor)
  t2 = s_f * c                     (GPSIMD tensor_tensor)
  c = 2*u + t2                     (DVE fused)
  sc = sigmoid(2*c)                (ACT)
  h' = (sc - 0.5) * s_o            (DVE fused)   [h' = h_true/2]

tanh is eliminated entirely via tanh(x) = 2*sigmoid(2x) - 1; the needed *2
factors are folded into host-side pre-scaled weights (w_hh rows *2 for the
h'=h/2 state convention, g-gate rows *2 again for the sigmoid-ized tanh),
so the gate ACT pass is a single plain sigmoid across all four gate banks.
"""

import numpy as np

import concourse.bass as bass
import concourse.mybir as mybir
import concourse.tile as tile
from concourse import bacc
from concourse.bass_utils import run_bass_kernel_spmd

B, S, D = 8, 2048, 128
H4 = 4 * D
W = 16
PAD = W - 1              # 15 zero-padded positions in front
CH = 512                 # positions per chunk (= one fp32 PSUM bank)
NCH = S // CH            # 4
XW = PAD + S + 1         # padded xT width (2064, kept even)
NT = S // D              # 16 transpose tiles

F32 = mybir.dt.float32
F32R = mybir.dt.float32r
SIG = mybir.ActivationFunctionType.Sigmoid
ADD = mybir.AluOpType.add
MUL = mybir.AluOpType.mult


def build_nc(mm_dtype=F32R, reps=1):
    nc = bacc.Bacc("TRN2")
    x_d = nc.dram_tensor("x", (S, D), F32, kind="ExternalInput")
    wih_d = nc.dram_tensor("wihT", (D, H4), F32, kind="ExternalInput")
    whh_d = nc.dram_tensor("whhT", (D, H4), F32, kind="ExternalInput")
    b_d = nc.dram_tensor("bcols", (D, 4), F32, kind="ExternalInput")
    id_d = nc.dram_tensor("ident", (D, D), F32R, kind="ExternalInput")
    y_d = nc.dram_tensor("y", (S, D), F32, kind="ExternalOutput")

    def r(ap):
        return ap

    with tile.TileContext(nc) as tc:
        with (
            tc.tile_pool(name="const", bufs=1) as cpool,
            tc.tile_pool(name="persist", bufs=1) as ppool,
            tc.tile_pool(name="state", bufs=1) as hpool,
            tc.tile_pool(name="work", bufs=3) as wpool,
            tc.tile_pool(name="io", bufs=1) as iopool,
        ):
            wih = cpool.tile([D, H4], F32R, name="wih")
            whh = cpool.tile([D, H4], F32R, name="whh")
            bc = cpool.tile([D, 4], F32, name="bc")
            ident = cpool.tile([D, D], F32R, name="ident")
            nc.sync.dma_start(out=wih, in_=wih_d.ap().bitcast(F32R))
            nc.sync.dma_start(out=whh, in_=whh_d.ap().bitcast(F32R))
            nc.sync.dma_start(out=bc, in_=b_d.ap())
            nc.sync.dma_start(out=ident, in_=id_d.ap())

            xn = iopool.tile([D, NT, D], F32, name="xn")
            nc.sync.dma_start(
                out=xn, in_=x_d.ap().rearrange("(n p) d -> p n d", p=D)
            )

            xT = ppool.tile([D, XW], F32R, name="xT")
            z16 = cpool.tile([D, 16], F32, name="z16")
            nc.vector.memset(z16, 0.0)
            nc.vector.tensor_copy(out=xT[:, 0:PAD], in_=z16[:, 0:PAD])
            nc.vector.tensor_copy(
                out=xT[:, PAD + S : XW], in_=z16[:, 0 : XW - PAD - S]
            )
            xg = [ppool.tile([D, XW], F32R, name=f"xg{j}") for j in range(4)]

            with tc.tile_pool(name="psum_setup", bufs=2, space="PSUM") as pset:
                for i in range(NT):
                    tp = pset.tile([D, D], F32, name="tp", tag="tp")
                    nc.tensor.transpose(tp, xn[:, i, :], ident.bitcast(F32))
                    nc.vector.tensor_copy(
                        out=xT[:, PAD + i * D : PAD + (i + 1) * D], in_=tp
                    )
                segs = [(k * CH, CH) for k in range(4)] + [(4 * CH, XW - 4 * CH)]
                for j in range(4):
                    for off, ln in segs:
                        pj = pset.tile([D, CH], F32, name="pj", tag="pj")
                        nc.tensor.matmul(
                            pj[:, :ln],
                            r(wih[:, j * D : (j + 1) * D]),
                            r(xT[:, off : off + ln]),
                            start=True,
                            stop=True,
                        )
                        nc.vector.tensor_scalar_add(
                            out=xg[j][:, off : off + ln],
                            in0=pj[:, :ln],
                            scalar1=bc[:, j : j + 1],
                        )

            h = [hpool.tile([D, CH], F32R, name=f"h{k}") for k in range(NCH)]
            c = [hpool.tile([D, CH], F32, name=f"c{k}") for k in range(NCH)]

            sig_insts = []
            hwr_insts = []
            with tc.tile_pool(name="psum_g", bufs=2, space="PSUM") as pgp:
                for w in [wi for _ in range(reps) for wi in range(W)]:
                    for k in range(NCH):
                        pg = pgp.tile([D, 4 * CH], F32, name="pg", tag="pg")
                        for j in range(4):
                            bank = pg[:, j * CH : (j + 1) * CH]
                            xsl = r(xg[j][:, k * CH + w : k * CH + w + CH])
                            if w > 0:
                                nc.tensor.matmul(
                                    bank,
                                    r(whh[:, j * D : (j + 1) * D]),
                                    r(h[k]),
                                    start=True,
                                    stop=False,
                                )
                                nc.tensor.matmul(
                                    bank, r(ident), xsl, start=False, stop=True
                                )
                            else:
                                nc.tensor.matmul(
                                    bank, r(ident), xsl, start=True, stop=True
                                )
                        s = wpool.tile([D, 4 * CH], F32, name="s", tag="s")
                        sig_insts.append(nc.scalar.activation(s, pg, SIG))
                        s_i = s[:, 0:CH]
                        s_f = s[:, CH : 2 * CH]
                        s_o = s[:, 2 * CH : 3 * CH]
                        s_g = s[:, 3 * CH : 4 * CH]
                        u = wpool.tile([D, CH], F32, name="u", tag="u")
                        nc.vector.scalar_tensor_tensor(u, s_g, -0.5, s_i, ADD, MUL)
                        if w > 0:
                            t2 = wpool.tile([D, CH], F32, name="t2", tag="t2")
                            nc.gpsimd.tensor_tensor(t2, s_f, c[k], MUL)
                            nc.vector.scalar_tensor_tensor(
                                c[k], u, 2.0, t2, MUL, ADD
                            )
                        else:
                            nc.vector.tensor_scalar_mul(c[k], u, 2.0)
                        sc = wpool.tile([D, CH], F32, name="sc", tag="sc")
                        nc.scalar.activation(sc, c[k], SIG, scale=2.0)
                        hwr_insts.append(
                            nc.vector.scalar_tensor_tensor(
                                h[k], sc, -0.5, s_o, ADD, MUL
                            )
                        )

            yt = iopool.tile([D, NT, D], F32, name="yt")
            with tc.tile_pool(name="psum_f", bufs=2, space="PSUM") as pfp:
                for i in range(NT):
                    tp = pfp.tile([D, D], F32, name="tpf", tag="tpf")
                    nc.tensor.transpose(
                        tp,
                        h[i // 4][:, (i % 4) * D : (i % 4 + 1) * D].bitcast(F32),
                        ident.bitcast(F32),
                    )
                    nc.vector.tensor_scalar_mul(yt[:, i, :], tp, 2.0)
            nc.sync.dma_start(
                out=y_d.ap().rearrange("(n p) d -> p n d", p=D), in_=yt
            )
    nc.compile()
    return nc


def prep_weights(w_ih, w_hh, b_ih, b_hh):
    """Gate-reorder to [i, f, o, g], fold both biases together, and pre-scale:
    w_hh rows *2 (h is stored as h/2), g-gate rows *2 again (tanh via
    sigmoid(2x))."""
    w_ih = np.asarray(w_ih, np.float32)
    w_hh = np.asarray(w_hh, np.float32)
    b = np.asarray(b_ih, np.float32) + np.asarray(b_hh, np.float32)
    perm = np.r_[0:128, 128:256, 384:512, 256:384]
    sc_ih = np.repeat(np.float32([1, 1, 1, 2]), D)
    sc_hh = np.repeat(np.float32([2, 2, 2, 4]), D)
    wihT = np.ascontiguousarray((w_ih[perm] * sc_ih[:, None]).T, np.float32)
    whhT = np.ascontiguousarray((w_hh[perm] * sc_hh[:, None]).T, np.float32)
    bcols = np.ascontiguousarray((b[perm] * sc_ih).reshape(4, D).T, np.float32)
    return wihT, whhT, bcols


_NC_CACHE = {}


def _get_nc(mm_dtype=F32R):
    key = str(mm_dtype)
    if key not in _NC_CACHE:
        _NC_CACHE[key] = build_nc(mm_dtype)
    return _NC_CACHE[key]


def run(x, w_ih, w_hh, b_ih, b_hh, trace=False, mm_dtype=F32R, **spmd_kwargs):
    x = np.ascontiguousarray(np.asarray(x, np.float32))
    assert x.shape == (B, S, D), x.shape
    wihT, whhT, bcols = prep_weights(w_ih, w_hh, b_ih, b_hh)
    nc = _get_nc(mm_dtype)
    ident = np.eye(D, dtype=np.float32)
    in_maps = [
        {"x": np.ascontiguousarray(x[cid]), "wihT": wihT, "whhT": whhT,
         "bcols": bcols, "ident": ident}
        for cid in range(B)
    ]
    res = run_bass_kernel_spmd(
        nc, in_maps, core_ids=list(range(B)), trace=trace, **spmd_kwargs
    )
    out = np.stack([res.results[cid]["y"] for cid in range(B)], axis=0)
    return out, res


def kernel(x, w_ih, w_hh, b_ih, b_hh, window_size):
    assert int(window_size) == W, window_size
    out, _ = run(x, w_ih, w_hh, b_ih, b_hh)
    return out


# revision 10
# speedup vs baseline: 1.5522x; 1.5522x over previous
"""LocalRNN (windowed LSTM) Trainium2 kernel.

Problem: x (8, 2048, 128); for every position s, run a W=16-step LSTM over
x[b, s-15 .. s] (zero-padded) with h0=c0=0; output the final hidden state.

Sharding: batch across the 8 cores (core c handles batch c; windows never
cross batches, so no halo is needed).

Per-core layout is feature-major ("transposed"): hidden dim d=128 lives on
SBUF partitions, positions on the free dim.  Per step and 512-position chunk:

  psum[d, 4*512] = whh_j @ h  (+)  I @ xg_j_slice      (fp32r matmuls, PSUM acc)
  s = sigmoid(psum)                                    (one ACT pass, 4 banks)
  u = (s_g - 0.5) * s_i            (DVE fused scalar_tensor_tensor)
  t2 = s_f * c                     (GPSIMD tensor_tensor)
  c = 2*u + t2                     (DVE fused)
  sc = sigmoid(2*c)                (ACT)
  h' = (sc - 0.5) * s_o            (DVE fused)   [h' = h_true/2]

tanh is eliminated entirely via tanh(x) = 2*sigmoid(2x) - 1; the needed *2
factors are folded into host-side pre-scaled weights (w_hh rows *2 for the
h'=h/2 state convention, g-gate rows *2 again for the sigmoid-ized tanh),
so the gate ACT pass is a single plain sigmoid across all four gate banks.
"""

import numpy as np

import concourse.bass as bass
import concourse.mybir as mybir
import concourse.tile as tile
from concourse import bacc
from concourse.bass_utils import run_bass_kernel_spmd

B, S, D = 8, 2048, 128
H4 = 4 * D
W = 16
PAD = W - 1              # 15 zero-padded positions in front
CH = 512                 # positions per chunk (= one fp32 PSUM bank)
NCH = S // CH            # 4
XW = PAD + S + 1         # padded xT width (2064, kept even)
NT = S // D              # 16 transpose tiles

F32 = mybir.dt.float32
F32R = mybir.dt.float32r
SIG = mybir.ActivationFunctionType.Sigmoid
ADD = mybir.AluOpType.add
MUL = mybir.AluOpType.mult


def build_nc(mm_dtype=F32R, reps=1, h_gpsimd=(1, 3), seg_outer=True,
             warm_table=True, group_mm=True, split_out_dma=4):
    nc = bacc.Bacc("TRN2")
    x_d = nc.dram_tensor("x", (S, D), F32, kind="ExternalInput")
    wih_d = nc.dram_tensor("wihT", (D, H4), F32, kind="ExternalInput")
    whh_d = nc.dram_tensor("whhT", (D, H4), F32, kind="ExternalInput")
    b_d = nc.dram_tensor("bcols", (D, 4), F32, kind="ExternalInput")
    id_d = nc.dram_tensor("ident", (D, D), F32R, kind="ExternalInput")
    y_d = nc.dram_tensor("y", (S, D), F32, kind="ExternalOutput")

    def r(ap):
        return ap

    with tile.TileContext(nc) as tc:
        with (
            tc.tile_pool(name="const", bufs=1) as cpool,
            tc.tile_pool(name="persist", bufs=1) as ppool,
            tc.tile_pool(name="state", bufs=1) as hpool,
            tc.tile_pool(name="work", bufs=3) as wpool,
            tc.tile_pool(name="io", bufs=1) as iopool,
        ):
            wih = cpool.tile([D, H4], F32R, name="wih")
            whh = cpool.tile([D, H4], F32R, name="whh")
            bc = cpool.tile([D, 4], F32, name="bc")
            ident = cpool.tile([D, D], F32R, name="ident")
            nc.sync.dma_start(out=wih, in_=wih_d.ap().bitcast(F32R))
            nc.sync.dma_start(out=whh, in_=whh_d.ap().bitcast(F32R))
            nc.sync.dma_start(out=bc, in_=b_d.ap())
            nc.sync.dma_start(out=ident, in_=id_d.ap())

            xn = iopool.tile([D, NT, D], F32, name="xn")
            nc.sync.dma_start(
                out=xn, in_=x_d.ap().rearrange("(n p) d -> p n d", p=D)
            )

            xT = ppool.tile([D, XW], F32R, name="xT")
            z16 = cpool.tile([D, 16], F32, name="z16")
            nc.vector.memset(z16, 0.0)
            if warm_table:
                zs = cpool.tile([D, 16], F32, name="zs")
                nc.scalar.activation(zs, z16, SIG)
            nc.vector.tensor_copy(out=xT[:, 0:PAD], in_=z16[:, 0:PAD])
            nc.vector.tensor_copy(
                out=xT[:, PAD + S : XW], in_=z16[:, 0 : XW - PAD - S]
            )
            xg = [ppool.tile([D, XW], F32R, name=f"xg{j}") for j in range(4)]

            with tc.tile_pool(name="psum_setup", bufs=2, space="PSUM") as pset:
                for i in range(NT):
                    tp = pset.tile([D, D], F32, name="tp", tag="tp")
                    nc.tensor.transpose(tp, xn[:, i, :], ident.bitcast(F32))
                    nc.vector.tensor_copy(
                        out=xT[:, PAD + i * D : PAD + (i + 1) * D], in_=tp
                    )
                segs = [(k * CH, CH) for k in range(4)] + [(4 * CH, XW - 4 * CH)]
                pairs = (
                    [(j, sg) for sg in segs for j in range(4)]
                    if seg_outer
                    else [(j, sg) for j in range(4) for sg in segs]
                )
                for j, (off, ln) in pairs:
                    pj = pset.tile([D, CH], F32, name="pj", tag="pj")
                    nc.tensor.matmul(
                        pj[:, :ln],
                        r(wih[:, j * D : (j + 1) * D]),
                        r(xT[:, off : off + ln]),
                        start=True,
                        stop=True,
                    )
                    nc.vector.tensor_scalar_add(
                        out=xg[j][:, off : off + ln],
                        in0=pj[:, :ln],
                        scalar1=bc[:, j : j + 1],
                    )

            h = [hpool.tile([D, CH], F32R, name=f"h{k}") for k in range(NCH)]
            c = [hpool.tile([D, CH], F32, name=f"c{k}") for k in range(NCH)]

            sig_insts = []
            hwr_insts = []
            with tc.tile_pool(name="psum_g", bufs=2, space="PSUM") as pgp:
                for w in [wi for _ in range(reps) for wi in range(W)]:
                    for k in range(NCH):
                        pg = pgp.tile([D, 4 * CH], F32, name="pg", tag="pg")
                        if w > 0 and group_mm:
                            for j in range(4):
                                nc.tensor.matmul(
                                    pg[:, j * CH : (j + 1) * CH],
                                    r(whh[:, j * D : (j + 1) * D]),
                                    r(h[k]),
                                    start=True,
                                    stop=False,
                                )
                            for j in range(4):
                                xsl = r(xg[j][:, k * CH + w : k * CH + w + CH])
                                nc.tensor.matmul(
                                    pg[:, j * CH : (j + 1) * CH],
                                    r(ident),
                                    xsl,
                                    start=False,
                                    stop=True,
                                    skip_group_check=True,
                                )
                        else:
                            for j in range(4):
                                bank = pg[:, j * CH : (j + 1) * CH]
                                xsl = r(xg[j][:, k * CH + w : k * CH + w + CH])
                                if w > 0:
                                    nc.tensor.matmul(
                                        bank,
                                        r(whh[:, j * D : (j + 1) * D]),
                                        r(h[k]),
                                        start=True,
                                        stop=False,
                                    )
                                    nc.tensor.matmul(
                                        bank, r(ident), xsl, start=False, stop=True
                                    )
                                else:
                                    nc.tensor.matmul(
                                        bank, r(ident), xsl, start=True, stop=True
                                    )
                        s = wpool.tile([D, 4 * CH], F32, name="s", tag="s")
                        sig_insts.append(nc.scalar.activation(s, pg, SIG))
                        s_i = s[:, 0:CH]
                        s_f = s[:, CH : 2 * CH]
                        s_o = s[:, 2 * CH : 3 * CH]
                        s_g = s[:, 3 * CH : 4 * CH]
                        u = wpool.tile([D, CH], F32, name="u", tag="u")
                        nc.vector.scalar_tensor_tensor(u, s_g, -0.5, s_i, ADD, MUL)
                        if w > 0:
                            t2 = wpool.tile([D, CH], F32, name="t2", tag="t2")
                            nc.gpsimd.tensor_tensor(t2, s_f, c[k], MUL)
                            nc.vector.scalar_tensor_tensor(
                                c[k], u, 2.0, t2, MUL, ADD
                            )
                        else:
                            nc.vector.tensor_scalar_mul(c[k], u, 2.0)
                        sc = wpool.tile([D, CH], F32, name="sc", tag="sc")
                        nc.scalar.activation(sc, c[k], SIG, scale=2.0)
                        h_eng = nc.gpsimd if k in h_gpsimd else nc.vector
                        hwr_insts.append(
                            h_eng.scalar_tensor_tensor(
                                h[k], sc, -0.5, s_o, ADD, MUL
                            )
                        )

            yt = iopool.tile([D, NT, D], F32, name="yt")
            with tc.tile_pool(name="psum_f", bufs=2, space="PSUM") as pfp:
                for i in range(NT):
                    tp = pfp.tile([D, D], F32, name="tpf", tag="tpf")
                    nc.tensor.transpose(
                        tp,
                        h[i // 4][:, (i % 4) * D : (i % 4 + 1) * D].bitcast(F32),
                        ident.bitcast(F32),
                    )
                    nc.vector.tensor_scalar_mul(yt[:, i, :], tp, 2.0)
            y_view = y_d.ap().rearrange("(n p) d -> p n d", p=D)
            nsplit = split_out_dma
            per = NT // nsplit
            for si in range(nsplit):
                nc.sync.dma_start(
                    out=y_view[:, si * per : (si + 1) * per, :],
                    in_=yt[:, si * per : (si + 1) * per, :],
                )
    nc.compile()
    return nc


def prep_weights(w_ih, w_hh, b_ih, b_hh):
    """Gate-reorder to [i, f, o, g], fold both biases together, and pre-scale:
    w_hh rows *2 (h is stored as h/2), g-gate rows *2 again (tanh via
    sigmoid(2x))."""
    w_ih = np.asarray(w_ih, np.float32)
    w_hh = np.asarray(w_hh, np.float32)
    b = np.asarray(b_ih, np.float32) + np.asarray(b_hh, np.float32)
    perm = np.r_[0:128, 128:256, 384:512, 256:384]
    sc_ih = np.repeat(np.float32([1, 1, 1, 2]), D)
    sc_hh = np.repeat(np.float32([2, 2, 2, 4]), D)
    wihT = np.ascontiguousarray((w_ih[perm] * sc_ih[:, None]).T, np.float32)
    whhT = np.ascontiguousarray((w_hh[perm] * sc_hh[:, None]).T, np.float32)
    bcols = np.ascontiguousarray((b[perm] * sc_ih).reshape(4, D).T, np.float32)
    return wihT, whhT, bcols


_NC_CACHE = {}


def _get_nc(mm_dtype=F32R):
    key = str(mm_dtype)
    if key not in _NC_CACHE:
        _NC_CACHE[key] = build_nc(mm_dtype)
    return _NC_CACHE[key]


def run(x, w_ih, w_hh, b_ih, b_hh, trace=False, mm_dtype=F32R, **spmd_kwargs):
    x = np.ascontiguousarray(np.asarray(x, np.float32))
    assert x.shape == (B, S, D), x.shape
    wihT, whhT, bcols = prep_weights(w_ih, w_hh, b_ih, b_hh)
    nc = _get_nc(mm_dtype)
    ident = np.eye(D, dtype=np.float32)
    in_maps = [
        {"x": np.ascontiguousarray(x[cid]), "wihT": wihT, "whhT": whhT,
         "bcols": bcols, "ident": ident}
        for cid in range(B)
    ]
    res = run_bass_kernel_spmd(
        nc, in_maps, core_ids=list(range(B)), trace=trace, **spmd_kwargs
    )
    out = np.stack([res.results[cid]["y"] for cid in range(B)], axis=0)
    return out, res


def kernel(x, w_ih, w_hh, b_ih, b_hh, window_size):
    assert int(window_size) == W, window_size
    out, _ = run(x, w_ih, w_hh, b_ih, b_hh)
    return out
